# revision 13
# baseline (speedup 1.0000x reference)
"""CLS-AttentionPool2d Trainium2 kernel (8 NeuronCores, data-parallel over batch).

v5 math (single CLS query => tiny attention), raw-x streaming:
  tokens[j] = x[b,:,j]                     (raw, native [C, HW] layout)
  mean      = tokens.mean(j);  cls = mean + pos0
  q  = (Wq @ cls + bq) / sqrt(C)
  qblk[k, (s,h)] = q_s[k] * [head(k) == h]
  m  = Wk.T @ qblk                         # per-head key-projected query
  scores[slot, j]   = qblk.T @ kp + m.T @ x     (kp = Wk @ pos.T, host-precomputed)
  scores[slot, cls] = m.T @ cls            (tiny N=1 matmuls per batch)
  e  = exp(scores); e0 = exp(scores_cls); Z = sum e + e0
  e' = e + e0/1024                         (folds CLS-mean into token weights)
  toks' = transpose(x) + posT              (posT add FUSED into the PSUM->SBUF
                                            copy after each PE transpose)
  w  = e'.T @ toks' + e0row @ pos0adjrow;  w *= 1/Z
  out = Wv @ w + bv                        (per-head block of Wv)

Scheduling: nothing gates on elementwise work -- transposes and means start
the moment each batch arrives (raw x). Waves {0-3},{4,5},{6,7} keep the
post-stream tail to a 2-batch chain. PE = transposes+scores+kp+w+qm,
DVE = fused tok copy-adds + half-means + smalls, ACT = half-means + exp.
"""

import math
import numpy as np

import concourse.bass as bass
import concourse.mybir as mybir
import concourse.tile as tile
from concourse import bacc
from concourse.bass import ts
from concourse.bass_utils import run_bass_kernel_spmd

F32 = mybir.dt.float32
BF16 = mybir.dt.bfloat16
AX = mybir.AxisListType
ALU = mybir.AluOpType
ACTF = mybir.ActivationFunctionType

B, C, HW = 64, 512, 1024
NH, DH = 8, 64
NCORES = 8
BPC = B // NCORES          # 8 batches per core
CT = C // 128              # 4 c-chunks
JT = HW // 128             # 8 j-chunks
ISQ = 1.0 / math.sqrt(C)

WAVES = [[0, 1, 2, 3], [4, 5], [6, 7]]
XBAR = set()               # batches whose x-transpose goes via DMA xbar

_CACHE = {}


def _build_nc():
    nc = bacc.Bacc("TRN2", target_bir_lowering=False, debug=False,
                   num_devices=NCORES)

    # ---- DRAM I/O ----
    xs = nc.dram_tensor("xs", [BPC, C, HW], F32, kind="ExternalInput")
    wqt = nc.dram_tensor("wqt", [128, CT, C], BF16, kind="ExternalInput")
    wk = nc.dram_tensor("wk", [128, CT, C], BF16, kind="ExternalInput")
    wvt = nc.dram_tensor("wvt", [128, CT, C], BF16, kind="ExternalInput")
    kp = nc.dram_tensor("kp", [128, CT, HW], BF16, kind="ExternalInput")
    postokt = nc.dram_tensor("postokt", [128, CT, JT, 128], BF16,
                             kind="ExternalInput")
    pos0row = nc.dram_tensor("pos0row", [1, C], BF16, kind="ExternalInput")
    pos0 = nc.dram_tensor("pos0", [128, CT], F32, kind="ExternalInput")
    bqs = nc.dram_tensor("bqs", [128, CT], F32, kind="ExternalInput")
    bv = nc.dram_tensor("bv", [128, CT], F32, kind="ExternalInput")
    mask32 = nc.dram_tensor("mask32", [128, CT, 32], F32, kind="ExternalInput")
    ident = nc.dram_tensor("ident", [128, 128], BF16, kind="ExternalInput")
    identf = nc.dram_tensor("identf", [128, 128], F32, kind="ExternalInput")
    out_d = nc.dram_tensor("out", [BPC, C], F32, kind="ExternalOutput")

    with tile.TileContext(nc) as tc:
        with (
            tc.tile_pool(name="persist", bufs=1) as pp,
            tc.tile_pool(name="big", bufs=1) as bigp,
            tc.tile_pool(name="work", bufs=3) as wp,
            tc.tile_pool(name="psA", bufs=1, space="PSUM") as psA,
            tc.tile_pool(name="psB", bufs=3, space="PSUM") as psB,
            tc.tile_pool(name="psC", bufs=1, space="PSUM") as psC,
            tc.tile_pool(name="psD", bufs=1, space="PSUM") as psD,
            tc.tile_pool(name="psE", bufs=1, space="PSUM") as psE,
        ):
            # ---- persistent tiles ----
            wqt_s = pp.tile([128, CT, C], BF16)
            wk_s = pp.tile([128, CT, C], BF16)
            wvt_s = pp.tile([128, CT, C], BF16)
            kp_s = pp.tile([128, CT, HW], BF16)
            postokt_s = pp.tile([128, CT, JT, 128], BF16)
            pos0row_s = pp.tile([1, C], BF16)
            pos0_s = pp.tile([128, CT], F32)
            bqs_s = pp.tile([128, CT], F32)
            bv_s = pp.tile([128, CT], F32)
            mask_s = pp.tile([128, CT, 32], F32)
            ident_s = pp.tile([128, 128], BF16)
            identf_s = pp.tile([128, 128], F32)
            sums = pp.tile([128, CT, BPC], F32)
            junk = pp.tile([128, HW], BF16)
            wt_all = pp.tile([128, CT, BPC, 8], BF16)

            # x loads go through gpsimd (only SWDGE DMAs cast f32->bf16),
            # split into two j-halves to keep the SWDGE ring pipelined,
            # with staggered issue so later batches don't steal bandwidth
            # from the first group's critical chain.
            xwaits = [0.0, 0.003, 0.006, 0.009, 0.022, 0.0275, 0.033, 0.0385]
            xb = []
            for i in range(BPC):
                xt = bigp.tile([128, CT, HW], BF16, tag="xb", bufs=8)
                xb.append(xt)
                src = xs[i].rearrange("(t p) j -> p t j", p=128)
                with tc.tile_wait_until(xwaits[i], enable=xwaits[i] > 0):
                    for h in range(2):
                        nc.gpsimd.dma_start(
                            out=xt[:, :, 512 * h:512 * (h + 1)],
                            in_=src[:, :, 512 * h:512 * (h + 1)])

            def emit_mean(i):
                # sums[:,:,i] = sum_j x (raw); chunks 0-1 on ACT, 2-3 on DVE
                for t in range(2):
                    nc.scalar.activation(
                        junk[:], xb[i][:, t, :], ACTF.Copy,
                        accum_out=sums[:, t, i:i + 1])
                nc.vector.tensor_reduce(
                    sums[:, 2:4, i:i + 1].rearrange("p t one -> p (t one)"),
                    xb[i][:, 2:4, :], axis=AX.X, op=ALU.add)

            # tables: early wave (means/q/m/scores), late wave (w/output)
            for dst, src in [(postokt_s, postokt), (wqt_s, wqt),
                             (wk_s, wk), (kp_s, kp),
                             (mask_s, mask32), (bqs_s, bqs),
                             (pos0_s, pos0), (ident_s, ident),
                             (identf_s, identf)]:
                nc.sync.dma_start(out=dst[:], in_=src[:])

            toks = [None] * BPC

            def emit_transpose(i):
                # toks[i] = transpose(x) + posT; the posT add is fused into
                # the PSUM->SBUF copy (tensor_tensor add on DVE)
                src = xb[i]
                tok = bigp.tile([128, CT, JT, 128], BF16, tag="tok", bufs=8)
                toks[i] = tok
                if i in XBAR:
                    nc.sync.dma_start_transpose(
                        tok[:].rearrange("p t j c -> p (t j) c"), src[:])
                    nc.vector.tensor_add(tok[:], tok[:], postokt_s[:])
                else:
                    for jp in range(JT // 2):
                        tp2 = psB.tile([128, CT, 2, 128], BF16, tag="psB")
                        for jj in range(2):
                            for t in range(CT):
                                nc.tensor.transpose(
                                    tp2[:, t, jj, :],
                                    src[:, t, ts(2 * jp + jj, 128)],
                                    ident_s[:])
                        nc.vector.tensor_add(
                            tok[:, :, 2 * jp:2 * jp + 2, :], tp2[:],
                            postokt_s[:, :, 2 * jp:2 * jp + 2, :])

            def phase_qm(wave):
                b0, ws = wave[0], len(wave)
                cls_all = wp.tile([128, CT, 4], BF16, tag="cls")
                cls_all = cls_all[:, :, 0:ws]
                nc.vector.scalar_tensor_tensor(
                    out=cls_all, in0=sums[:, :, b0:b0 + ws],
                    scalar=1.0 / HW,
                    in1=pos0_s[:, :, None].broadcast_to([128, CT, ws]),
                    op0=ALU.mult, op1=ALU.add)

                q_ps = psC.tile([128, CT, 4], F32, tag="psC")
                for mc in range(CT):
                    for tk in range(CT):
                        nc.tensor.matmul(
                            q_ps[:, mc, 0:ws], wqt_s[:, tk, ts(mc, 128)],
                            cls_all[:, tk, :],
                            start=(tk == 0), stop=(tk == CT - 1))
                q_sb = wp.tile([128, CT, 4], F32, tag="qsb")
                q_sb = q_sb[:, :, 0:ws]
                nc.vector.scalar_tensor_tensor(
                    out=q_sb, in0=q_ps[:, :, 0:ws], scalar=ISQ,
                    in1=bqs_s[:, :, None].broadcast_to([128, CT, ws]),
                    op0=ALU.mult, op1=ALU.add)

                qblk = wp.tile([128, CT, 4, 32], BF16, tag="qblk")
                qblk = qblk[:, :, 0:ws, :]
                nc.vector.tensor_mul(
                    qblk,
                    q_sb[:, :, :, None].broadcast_to([128, CT, ws, 32]),
                    mask_s[:, :, None, :].broadcast_to([128, CT, ws, 32]))
                qblk_f = qblk.rearrange("p t s u -> p t (s u)")

                m_ps = psC.tile([128, CT, 128], F32, tag="psC")
                for mc in range(CT):
                    for tk in range(CT):
                        nc.tensor.matmul(
                            m_ps[:, mc, 0:32 * ws], wk_s[:, tk, ts(mc, 128)],
                            qblk_f[:, tk, :],
                            start=(tk == 0), stop=(tk == CT - 1))
                m_sb = wp.tile([128, CT, 128], BF16, tag="msb")
                m_sb = m_sb[:, :, 0:32 * ws]
                nc.vector.tensor_copy(m_sb, m_ps[:, :, 0:32 * ws])
                return cls_all, m_sb, qblk_f

            def phase_scores(wave, cls_all, m_sb, qblk_f):
                ws = len(wave)
                sc_ps = psA.tile([128, 2, 512], F32, tag="psA")
                cls_ps = psE.tile([128, 1], F32, tag="psE")
                # kp term first (start=True): scores += qblk.T @ kp
                for tk in range(CT):
                    for jc in range(2):
                        nc.tensor.matmul(
                            sc_ps[0:32 * ws, jc, :], qblk_f[:, tk, :],
                            kp_s[:, tk, ts(jc, 512)],
                            start=(tk == 0), stop=False,
                            skip_group_check=True)
                for s, b in enumerate(wave):
                    for tk in range(CT):
                        for jc in range(2):
                            nc.tensor.matmul(
                                sc_ps[32 * s:32 * s + 8, jc, :],
                                m_sb[:, tk, 32 * s:32 * s + 8],
                                xb[b][:, tk, ts(jc, 512)],
                                start=False, stop=(tk == CT - 1),
                                tile_position=(0, 32 * s),
                                skip_group_check=True)
                        nc.tensor.matmul(
                            cls_ps[32 * s:32 * s + 8, :],
                            m_sb[:, tk, 32 * s:32 * s + 8],
                            cls_all[:, tk, s:s + 1],
                            start=(tk == 0), stop=(tk == CT - 1),
                            tile_position=(0, 32 * s),
                            skip_group_check=True)
                return sc_ps, cls_ps

            def phase_softmax(wave, sc_ps, cls_ps):
                p_sb = wp.tile([128, HW], BF16, tag="psb")
                sumexp = wp.tile([128, 1], F32, tag="sumexp")
                e0f = wp.tile([128, 1], F32, tag="e0f")
                nc.scalar.activation(p_sb[:], sc_ps[:, 0:2, :],
                                     ACTF.Exp, scale=1.0, accum_out=sumexp[:])
                nc.scalar.activation(e0f[:], cls_ps[:], ACTF.Exp, scale=1.0)
                nc.vector.tensor_add(sumexp[:], sumexp[:], e0f[:])
                rz = wp.tile([128, 1], F32, tag="rz")
                nc.vector.reciprocal(rz[:], sumexp[:])
                e0s = wp.tile([128, 1], F32, tag="e0s")
                nc.vector.tensor_scalar_mul(e0s[:], e0f[:], 1.0 / HW)
                nc.vector.tensor_scalar_add(p_sb[:], p_sb[:], e0s[:])
                # e0row: [1, 128] row of raw e0 per slot, for the pos0adj term
                tp_e0 = psE.tile([1, 128], F32, tag="psE")
                nc.tensor.transpose(tp_e0[:], e0f[:], identf_s[:])
                e0row = wp.tile([1, 128], BF16, tag="e0row")
                nc.vector.tensor_copy(e0row[:], tp_e0[:])
                return p_sb, rz, e0row

            def phase_pT(wave, p_sb):
                pT = wp.tile([128, JT, 128], BF16, tag="pT")
                for half in range(2):
                    tp = psB.tile([128, 4, 128], BF16, tag="psB")
                    for k in range(4):
                        jc = half * 4 + k
                        nc.tensor.transpose(tp[:, k, :], p_sb[:, ts(jc, 128)],
                                            ident_s[:])
                    nc.vector.tensor_copy(
                        pT[:, half * 4:(half + 1) * 4, :], tp[:])
                return pT

            def phase_w(wave, pT, e0row, rz):
                ws = len(wave)
                w_ps = psD.tile([128, C], F32, tag="psD")
                for s, b in enumerate(wave):
                    for jc in range(JT):
                        nc.tensor.matmul(
                            w_ps[32 * s:32 * s + 8, :],
                            pT[:, jc, 32 * s:32 * s + 8],
                            toks[b][:, :, jc, :],
                            start=(jc == 0), stop=False,
                            tile_position=(0, 32 * s),
                            skip_group_check=True)
                nc.tensor.matmul(w_ps[:], e0row[:], pos0row_s[:],
                                 start=False, stop=True,
                                 skip_group_check=True)
                w_sb = wp.tile([128, C], BF16, tag="wsb")
                nc.vector.tensor_scalar_mul(w_sb[:], w_ps[:], rz[:])

                tp3 = psB.tile([128, CT, 128], BF16, tag="psB")
                for mc in range(CT):
                    nc.tensor.transpose(tp3[:, mc, :],
                                        w_sb[:, ts(mc, 128)], ident_s[:])
                nc.vector.tensor_copy(
                    wt_all[:, :, wave[0]:wave[0] + ws, :],
                    tp3[:].rearrange("p t (s u) -> p t s u", s=4)[
                        :, :, 0:ws, 0:8])

            def run_wave(wave):
                cls_all, m_sb, qblk_f = phase_qm(wave)
                sc, cps = phase_scores(wave, cls_all, m_sb, qblk_f)
                p, rz, e0row = phase_softmax(wave, sc, cps)
                pT = phase_pT(wave, p)
                return pT, e0row, rz

            # ================== schedule ==================
            for i in range(4):
                emit_mean(i)
                emit_transpose(i)
            # late tables: needed from phase_w(wave 0) onward
            for dst, src in [(wvt_s, wvt),
                             (pos0row_s, pos0row), (bv_s, bv)]:
                nc.sync.dma_start(out=dst[:], in_=src[:])
            pT0, e0row0, rz0 = run_wave(WAVES[0])
            with tc.tile_wait_until(0.024):
                emit_mean(4)
                emit_transpose(4)
            with tc.tile_wait_until(0.029):
                emit_mean(5)
                emit_transpose(5)
            phase_w(WAVES[0], pT0, e0row0, rz0)
            with tc.tile_wait_until(0.035):
                emit_mean(6)
                emit_transpose(6)
            with tc.tile_wait_until(0.041):
                emit_mean(7)
                emit_transpose(7)
            pT1, e0row1, rz1 = run_wave(WAVES[1])
            phase_w(WAVES[1], pT1, e0row1, rz1)
            pT2, e0row2, rz2 = run_wave(WAVES[2])
            phase_w(WAVES[2], pT2, e0row2, rz2)

            # ---------------- output projection (all 8 batches) ----------
            out_ps = psC.tile([128, CT, BPC], F32, tag="psC")
            for h in range(NH):
                pr, hi = h // 2, 64 * (h % 2)
                for tk in range(CT):
                    nc.tensor.matmul(
                        out_ps[hi:hi + 64, pr, :],
                        wvt_s[:, tk, h * DH:(h + 1) * DH],
                        wt_all[:, tk, :, h],
                        start=(tk == 0), stop=(tk == CT - 1),
                        tile_position=(0, hi),
                        skip_group_check=True)
            out_sb = wp.tile([128, CT, BPC], F32, tag="outsb")
            for pr in range(CT):
                nc.vector.tensor_scalar_add(out_sb[:, pr, :],
                                            out_ps[:, pr, :],
                                            bv_s[:, pr:pr + 1])
            # transpose to [batch, c] so all 8 rows store as one DMA of
            # contiguous 2KB lines
            oT_ps = psB.tile([8, CT, 128], F32, tag="psB")
            for mc in range(CT):
                nc.tensor.transpose(oT_ps[:, mc, :], out_sb[:, mc, :],
                                    identf_s[:])
            oT_sb = wp.tile([8, CT, 128], F32, tag="oTsb")
            nc.vector.tensor_copy(oT_sb[:], oT_ps[:])
            nc.sync.dma_start(
                out=out_d[:].rearrange("b (t c) -> b t c", t=CT),
                in_=oT_sb[:])

    nc.compile()
    return nc


def _prep(pos_emb, Wq, bq, Wk, bk, Wv, bv):
    import ml_dtypes
    bf = ml_dtypes.bfloat16

    def ptn(v):  # [512] -> [128, CT], c = t*128 + p
        return np.ascontiguousarray(v.reshape(CT, 128).T)

    def chunkk(w):  # [512, N] -> [128, CT, N], k = t*128 + p
        return np.ascontiguousarray(w.reshape(CT, 128, -1).transpose(1, 0, 2))

    p1 = pos_emb[1:].sum(axis=0)
    pos0adj = pos_emb[0] - p1 / HW
    mask = np.zeros((128, CT, 32), np.float32)
    for p in range(128):
        for t in range(CT):
            h = (t * 128 + p) // DH
            mask[p, t, h] = 1.0

    kpm = Wk.astype(np.float64) @ pos_emb[1:].astype(np.float64).T  # [C, HW]
    # postokt[j2, t, jt, c_lo] = pos_emb[1 + jt*128 + j2, t*128 + c_lo]
    postokt = np.ascontiguousarray(
        pos_emb[1:].reshape(JT, 128, CT, 128).transpose(1, 2, 0, 3))
    return {
        "wqt": chunkk(np.ascontiguousarray(Wq.T)).astype(bf),
        "wk": chunkk(Wk).astype(bf),
        "wvt": chunkk(np.ascontiguousarray(Wv.T)).astype(bf),
        "kp": chunkk(kpm.astype(np.float32)).astype(bf),
        "postokt": postokt.astype(bf),
        "pos0row": np.ascontiguousarray(pos0adj.reshape(1, C)).astype(bf),
        "pos0": ptn(pos_emb[0]),
        "bqs": ptn(bq * ISQ),
        "bv": np.ascontiguousarray(bv.reshape(CT, 128).T),
        "mask32": mask,
        "ident": np.eye(128, dtype=np.float32).astype(bf),
        "identf": np.eye(128, dtype=np.float32),
    }


def kernel(x, pos_emb, Wq, bq, Wk, bk, Wv, bv, num_heads):
    assert int(num_heads) == NH
    x = np.asarray(x, dtype=np.float32).reshape(B, C, HW)
    if "nc" not in _CACHE:
        _CACHE["nc"] = _build_nc()
    nc = _CACHE["nc"]
    shared = _prep(np.asarray(pos_emb, np.float32), np.asarray(Wq, np.float32),
                   np.asarray(bq, np.float32), np.asarray(Wk, np.float32),
                   np.asarray(bk, np.float32), np.asarray(Wv, np.float32),
                   np.asarray(bv, np.float32))
    in_maps = []
    for i in range(NCORES):
        m = dict(shared)
        m["xs"] = np.ascontiguousarray(x[i * BPC:(i + 1) * BPC])
        in_maps.append(m)
    res = run_bass_kernel_spmd(nc, in_maps, list(range(NCORES)))
    out = np.concatenate([res.results[i]["out"] for i in range(NCORES)], axis=0)
    return out.astype(np.float32)


# revision 14
# speedup vs baseline: 1.0174x; 1.0174x over previous
"""CLS-AttentionPool2d Trainium2 kernel (8 NeuronCores, data-parallel over batch).

v5 math (single CLS query => tiny attention), raw-x streaming:
  tokens[j] = x[b,:,j]                     (raw, native [C, HW] layout)
  mean      = tokens.mean(j);  cls = mean + pos0
  q  = (Wq @ cls + bq) / sqrt(C)
  qblk[k, (s,h)] = q_s[k] * [head(k) == h]
  m  = Wk.T @ qblk                         # per-head key-projected query
  scores[slot, j]   = qblk.T @ kp + m.T @ x     (kp = Wk @ pos.T, host-precomputed)
  scores[slot, cls] = m.T @ cls            (tiny N=1 matmuls per batch)
  e  = exp(scores); e0 = exp(scores_cls); Z = sum e + e0
  e' = e + e0/1024                         (folds CLS-mean into token weights)
  toks' = transpose(x) + posT              (posT add FUSED into the PSUM->SBUF
                                            copy after each PE transpose)
  w  = e'.T @ toks' + e0row @ pos0adjrow;  w *= 1/Z
  out = Wv @ w + bv                        (per-head block of Wv)

Scheduling: nothing gates on elementwise work -- transposes and means start
the moment each batch arrives (raw x). Waves {0-3},{4,5},{6,7} keep the
post-stream tail to a 2-batch chain. PE = transposes+scores+kp+w+qm,
DVE = fused tok copy-adds + half-means + smalls, ACT = half-means + exp.
"""

import math
import numpy as np

import concourse.bass as bass
import concourse.mybir as mybir
import concourse.tile as tile
from concourse import bacc
from concourse.bass import ts
from concourse.bass_utils import run_bass_kernel_spmd

F32 = mybir.dt.float32
BF16 = mybir.dt.bfloat16
AX = mybir.AxisListType
ALU = mybir.AluOpType
ACTF = mybir.ActivationFunctionType

B, C, HW = 64, 512, 1024
NH, DH = 8, 64
NCORES = 8
BPC = B // NCORES          # 8 batches per core
CT = C // 128              # 4 c-chunks
JT = HW // 128             # 8 j-chunks
ISQ = 1.0 / math.sqrt(C)

WAVES = [[0, 1, 2, 3], [4, 5], [6, 7]]
XBAR = {6, 7}              # batches whose x-transpose goes via DMA xbar

_CACHE = {}


def _build_nc():
    nc = bacc.Bacc("TRN2", target_bir_lowering=False, debug=False,
                   num_devices=NCORES)

    # ---- DRAM I/O ----
    xs = nc.dram_tensor("xs", [BPC, C, HW], F32, kind="ExternalInput")
    wqt = nc.dram_tensor("wqt", [128, CT, C], BF16, kind="ExternalInput")
    wk = nc.dram_tensor("wk", [128, CT, C], BF16, kind="ExternalInput")
    wvt = nc.dram_tensor("wvt", [128, CT, C], BF16, kind="ExternalInput")
    kp = nc.dram_tensor("kp", [128, CT, HW], BF16, kind="ExternalInput")
    postok = nc.dram_tensor("postok", [128, JT, C], BF16,
                            kind="ExternalInput")
    pos0row = nc.dram_tensor("pos0row", [1, C], BF16, kind="ExternalInput")
    pos0 = nc.dram_tensor("pos0", [128, CT], F32, kind="ExternalInput")
    bqs = nc.dram_tensor("bqs", [128, CT], F32, kind="ExternalInput")
    bv = nc.dram_tensor("bv", [128, CT], F32, kind="ExternalInput")
    mask32 = nc.dram_tensor("mask32", [128, CT, 32], F32, kind="ExternalInput")
    ident = nc.dram_tensor("ident", [128, 128], BF16, kind="ExternalInput")
    identf = nc.dram_tensor("identf", [128, 128], F32, kind="ExternalInput")
    out_d = nc.dram_tensor("out", [BPC, C], F32, kind="ExternalOutput")

    with tile.TileContext(nc) as tc:
        with (
            tc.tile_pool(name="persist", bufs=1) as pp,
            tc.tile_pool(name="big", bufs=1) as bigp,
            tc.tile_pool(name="work", bufs=3) as wp,
            tc.tile_pool(name="psA", bufs=1, space="PSUM") as psA,
            tc.tile_pool(name="psB", bufs=3, space="PSUM") as psB,
            tc.tile_pool(name="psC", bufs=1, space="PSUM") as psC,
            tc.tile_pool(name="psD", bufs=1, space="PSUM") as psD,
            tc.tile_pool(name="psE", bufs=1, space="PSUM") as psE,
        ):
            # ---- persistent tiles ----
            wqt_s = pp.tile([128, CT, C], BF16)
            wk_s = pp.tile([128, CT, C], BF16)
            wvt_s = pp.tile([128, CT, C], BF16)
            kp_s = pp.tile([128, CT, HW], BF16)
            postok_s = pp.tile([128, JT, C], BF16)
            pos0row_s = pp.tile([1, C], BF16)
            pos0_s = pp.tile([128, CT], F32)
            bqs_s = pp.tile([128, CT], F32)
            bv_s = pp.tile([128, CT], F32)
            mask_s = pp.tile([128, CT, 32], F32)
            ident_s = pp.tile([128, 128], BF16)
            identf_s = pp.tile([128, 128], F32)
            sums = pp.tile([128, CT, BPC], F32)
            junk = pp.tile([128, HW], BF16)
            wt_all = pp.tile([128, CT, BPC, 8], BF16)

            # x loads go through gpsimd (only SWDGE DMAs cast f32->bf16),
            # split into two j-halves to keep the SWDGE ring pipelined,
            # with staggered issue so later batches don't steal bandwidth
            # from the first group's critical chain.
            xwaits = [0.0, 0.003, 0.006, 0.009, 0.022, 0.0275, 0.033, 0.0385]
            xb = []
            for i in range(BPC):
                xt = bigp.tile([128, CT, HW], BF16, tag="xb", bufs=8)
                xb.append(xt)
                src = xs[i].rearrange("(t p) j -> p t j", p=128)
                with tc.tile_wait_until(xwaits[i], enable=xwaits[i] > 0):
                    for h in range(2):
                        nc.gpsimd.dma_start(
                            out=xt[:, :, 512 * h:512 * (h + 1)],
                            in_=src[:, :, 512 * h:512 * (h + 1)])

            def emit_mean(i):
                # sums[:,:,i] = sum_j x (raw), one DVE reduce
                nc.vector.tensor_reduce(
                    sums[:, :, i:i + 1].rearrange("p t one -> p (t one)"),
                    xb[i][:], axis=AX.X, op=ALU.add)

            # tables: early wave (means/q/m/scores), late wave (w/output)
            for dst, src in [(wqt_s, wqt),
                             (wk_s, wk), (kp_s, kp),
                             (mask_s, mask32), (bqs_s, bqs),
                             (pos0_s, pos0), (ident_s, ident),
                             (identf_s, identf)]:
                nc.sync.dma_start(out=dst[:], in_=src[:])

            toks = [None] * BPC

            def emit_transpose(i):
                # toks[i] = transpose(x) + posT; the posT add is fused into
                # the PSUM->SBUF copy (tensor_tensor add on DVE)
                src = xb[i]
                tok = bigp.tile([128, CT, JT, 128], BF16, tag="tok", bufs=8)
                toks[i] = tok
                if i in XBAR:
                    nc.sync.dma_start_transpose(
                        tok[:].rearrange("p t j c -> p (t j) c"), src[:])
                else:
                    for jp in range(JT // 2):
                        tp2 = psB.tile([128, CT, 2, 128], BF16, tag="psB")
                        for jj in range(2):
                            for t in range(CT):
                                nc.tensor.transpose(
                                    tp2[:, t, jj, :],
                                    src[:, t, ts(2 * jp + jj, 128)],
                                    ident_s[:])
                        if jp % 2 == 1:
                            nc.scalar.copy(tok[:, :, 2 * jp:2 * jp + 2, :],
                                           tp2[:])
                        else:
                            nc.vector.tensor_copy(
                                tok[:, :, 2 * jp:2 * jp + 2, :], tp2[:])

            def phase_qm(wave):
                b0, ws = wave[0], len(wave)
                cls_all = wp.tile([128, CT, 4], BF16, tag="cls")
                cls_all = cls_all[:, :, 0:ws]
                nc.vector.scalar_tensor_tensor(
                    out=cls_all, in0=sums[:, :, b0:b0 + ws],
                    scalar=1.0 / HW,
                    in1=pos0_s[:, :, None].broadcast_to([128, CT, ws]),
                    op0=ALU.mult, op1=ALU.add)

                q_ps = psC.tile([128, CT, 4], F32, tag="psC")
                for mc in range(CT):
                    for tk in range(CT):
                        nc.tensor.matmul(
                            q_ps[:, mc, 0:ws], wqt_s[:, tk, ts(mc, 128)],
                            cls_all[:, tk, :],
                            start=(tk == 0), stop=(tk == CT - 1))
                q_sb = wp.tile([128, CT, 4], F32, tag="qsb")
                q_sb = q_sb[:, :, 0:ws]
                nc.vector.scalar_tensor_tensor(
                    out=q_sb, in0=q_ps[:, :, 0:ws], scalar=ISQ,
                    in1=bqs_s[:, :, None].broadcast_to([128, CT, ws]),
                    op0=ALU.mult, op1=ALU.add)

                qblk = wp.tile([128, CT, 4, 32], BF16, tag="qblk")
                qblk = qblk[:, :, 0:ws, :]
                nc.vector.tensor_mul(
                    qblk,
                    q_sb[:, :, :, None].broadcast_to([128, CT, ws, 32]),
                    mask_s[:, :, None, :].broadcast_to([128, CT, ws, 32]))
                qblk_f = qblk.rearrange("p t s u -> p t (s u)")

                m_ps = psC.tile([128, CT, 128], F32, tag="psC")
                for mc in range(CT):
                    for tk in range(CT):
                        nc.tensor.matmul(
                            m_ps[:, mc, 0:32 * ws], wk_s[:, tk, ts(mc, 128)],
                            qblk_f[:, tk, :],
                            start=(tk == 0), stop=(tk == CT - 1))
                m_sb = wp.tile([128, CT, 128], BF16, tag="msb")
                m_sb = m_sb[:, :, 0:32 * ws]
                nc.vector.tensor_copy(m_sb, m_ps[:, :, 0:32 * ws])
                return cls_all, m_sb, qblk_f

            def phase_scores(wave, cls_all, m_sb, qblk_f):
                ws = len(wave)
                sc_ps = psA.tile([128, 2, 512], F32, tag="psA")
                cls_ps = psE.tile([128, 1], F32, tag="psE")
                # kp term first (start=True): scores += qblk.T @ kp
                for tk in range(CT):
                    for jc in range(2):
                        nc.tensor.matmul(
                            sc_ps[0:32 * ws, jc, :], qblk_f[:, tk, :],
                            kp_s[:, tk, ts(jc, 512)],
                            start=(tk == 0), stop=False,
                            skip_group_check=True)
                for s, b in enumerate(wave):
                    for tk in range(CT):
                        for jc in range(2):
                            nc.tensor.matmul(
                                sc_ps[32 * s:32 * s + 8, jc, :],
                                m_sb[:, tk, 32 * s:32 * s + 8],
                                xb[b][:, tk, ts(jc, 512)],
                                start=False, stop=(tk == CT - 1),
                                tile_position=(0, 32 * s),
                                skip_group_check=True)
                        nc.tensor.matmul(
                            cls_ps[32 * s:32 * s + 8, :],
                            m_sb[:, tk, 32 * s:32 * s + 8],
                            cls_all[:, tk, s:s + 1],
                            start=(tk == 0), stop=(tk == CT - 1),
                            tile_position=(0, 32 * s),
                            skip_group_check=True)
                return sc_ps, cls_ps

            def phase_softmax(wave, sc_ps, cls_ps):
                p_sb = wp.tile([128, HW], BF16, tag="psb")
                sumexp = wp.tile([128, 1], F32, tag="sumexp")
                e0f = wp.tile([128, 1], F32, tag="e0f")
                nc.scalar.activation(p_sb[:], sc_ps[:, 0:2, :],
                                     ACTF.Exp, scale=1.0, accum_out=sumexp[:])
                nc.scalar.activation(e0f[:], cls_ps[:], ACTF.Exp, scale=1.0)
                nc.vector.tensor_add(sumexp[:], sumexp[:], e0f[:])
                rz = wp.tile([128, 1], F32, tag="rz")
                nc.vector.reciprocal(rz[:], sumexp[:])
                e0s = wp.tile([128, 1], F32, tag="e0s")
                nc.vector.tensor_scalar_mul(e0s[:], e0f[:], 1.0 / HW)
                nc.vector.tensor_scalar_add(p_sb[:], p_sb[:], e0s[:])
                # e0row: [1, 128] row of raw e0 per slot, for the pos0adj term
                tp_e0 = psE.tile([1, 128], F32, tag="psE")
                nc.tensor.transpose(tp_e0[:], e0f[:], identf_s[:])
                e0row = wp.tile([1, 128], BF16, tag="e0row")
                nc.vector.tensor_copy(e0row[:], tp_e0[:])
                return p_sb, rz, e0row

            def phase_pT(wave, p_sb):
                pT = wp.tile([128, JT, 128], BF16, tag="pT")
                for half in range(2):
                    tp = psB.tile([128, 4, 128], BF16, tag="psB")
                    for k in range(4):
                        jc = half * 4 + k
                        nc.tensor.transpose(tp[:, k, :], p_sb[:, ts(jc, 128)],
                                            ident_s[:])
                    nc.vector.tensor_copy(
                        pT[:, half * 4:(half + 1) * 4, :], tp[:])
                return pT

            def phase_w(wave, pT, e0row, rz):
                ws = len(wave)
                w_ps = psD.tile([128, C], F32, tag="psD")
                for s, b in enumerate(wave):
                    for jc in range(JT):
                        nc.tensor.matmul(
                            w_ps[32 * s:32 * s + 8, :],
                            pT[:, jc, 32 * s:32 * s + 8],
                            toks[b][:, :, jc, :],
                            start=(jc == 0), stop=False,
                            tile_position=(0, 32 * s),
                            skip_group_check=True)
                for jc in range(JT):
                    nc.tensor.matmul(w_ps[:], pT[:, jc, :],
                                     postok_s[:, jc, :],
                                     start=False, stop=False,
                                     skip_group_check=True)
                nc.tensor.matmul(w_ps[:], e0row[:], pos0row_s[:],
                                 start=False, stop=True,
                                 skip_group_check=True)
                w_sb = wp.tile([128, C], BF16, tag="wsb")
                nc.vector.tensor_scalar_mul(w_sb[:], w_ps[:], rz[:])

                tp3 = psB.tile([128, CT, 128], BF16, tag="psB")
                for mc in range(CT):
                    nc.tensor.transpose(tp3[:, mc, :],
                                        w_sb[:, ts(mc, 128)], ident_s[:])
                nc.vector.tensor_copy(
                    wt_all[:, :, wave[0]:wave[0] + ws, :],
                    tp3[:].rearrange("p t (s u) -> p t s u", s=4)[
                        :, :, 0:ws, 0:8])

            def run_wave(wave):
                cls_all, m_sb, qblk_f = phase_qm(wave)
                sc, cps = phase_scores(wave, cls_all, m_sb, qblk_f)
                p, rz, e0row = phase_softmax(wave, sc, cps)
                pT = phase_pT(wave, p)
                return pT, e0row, rz

            # ================== schedule ==================
            for i in range(4):
                emit_mean(i)
                emit_transpose(i)
            # late tables: needed from phase_w(wave 0) onward
            for dst, src in [(wvt_s, wvt), (postok_s, postok),
                             (pos0row_s, pos0row), (bv_s, bv)]:
                nc.sync.dma_start(out=dst[:], in_=src[:])
            pT0, e0row0, rz0 = run_wave(WAVES[0])
            with tc.tile_wait_until(0.024):
                emit_mean(4)
                emit_transpose(4)
            with tc.tile_wait_until(0.029):
                emit_mean(5)
                emit_transpose(5)
            phase_w(WAVES[0], pT0, e0row0, rz0)
            with tc.tile_wait_until(0.035):
                emit_mean(6)
                emit_transpose(6)
            with tc.tile_wait_until(0.041):
                emit_mean(7)
                emit_transpose(7)
            pT1, e0row1, rz1 = run_wave(WAVES[1])
            phase_w(WAVES[1], pT1, e0row1, rz1)
            pT2, e0row2, rz2 = run_wave(WAVES[2])
            phase_w(WAVES[2], pT2, e0row2, rz2)

            # ---------------- output projection (all 8 batches) ----------
            out_ps = psC.tile([128, CT, BPC], F32, tag="psC")
            for h in range(NH):
                pr, hi = h // 2, 64 * (h % 2)
                for tk in range(CT):
                    nc.tensor.matmul(
                        out_ps[hi:hi + 64, pr, :],
                        wvt_s[:, tk, h * DH:(h + 1) * DH],
                        wt_all[:, tk, :, h],
                        start=(tk == 0), stop=(tk == CT - 1),
                        tile_position=(0, hi),
                        skip_group_check=True)
            out_sb = wp.tile([128, CT, BPC], F32, tag="outsb")
            for pr in range(CT):
                nc.vector.tensor_scalar_add(out_sb[:, pr, :],
                                            out_ps[:, pr, :],
                                            bv_s[:, pr:pr + 1])
            # transpose to [batch, c] so all 8 rows store as one DMA of
            # contiguous 2KB lines
            oT_ps = psB.tile([8, CT, 128], F32, tag="psB")
            for mc in range(CT):
                nc.tensor.transpose(oT_ps[:, mc, :], out_sb[:, mc, :],
                                    identf_s[:])
            oT_sb = wp.tile([8, CT, 128], F32, tag="oTsb")
            nc.vector.tensor_copy(oT_sb[:], oT_ps[:])
            nc.sync.dma_start(
                out=out_d[:].rearrange("b (t c) -> b t c", t=CT),
                in_=oT_sb[:])

    nc.compile()
    return nc


def _prep(pos_emb, Wq, bq, Wk, bk, Wv, bv):
    import ml_dtypes
    bf = ml_dtypes.bfloat16

    def ptn(v):  # [512] -> [128, CT], c = t*128 + p
        return np.ascontiguousarray(v.reshape(CT, 128).T)

    def chunkk(w):  # [512, N] -> [128, CT, N], k = t*128 + p
        return np.ascontiguousarray(w.reshape(CT, 128, -1).transpose(1, 0, 2))

    p1 = pos_emb[1:].sum(axis=0)
    pos0adj = pos_emb[0] - p1 / HW
    mask = np.zeros((128, CT, 32), np.float32)
    for p in range(128):
        for t in range(CT):
            h = (t * 128 + p) // DH
            mask[p, t, h] = 1.0

    kpm = Wk.astype(np.float64) @ pos_emb[1:].astype(np.float64).T  # [C, HW]
    postok_nat = np.ascontiguousarray(
        pos_emb[1:].reshape(JT, 128, C).transpose(1, 0, 2))
    return {
        "wqt": chunkk(np.ascontiguousarray(Wq.T)).astype(bf),
        "wk": chunkk(Wk).astype(bf),
        "wvt": chunkk(np.ascontiguousarray(Wv.T)).astype(bf),
        "kp": chunkk(kpm.astype(np.float32)).astype(bf),
        "postok": postok_nat.astype(bf),
        "pos0row": np.ascontiguousarray(pos0adj.reshape(1, C)).astype(bf),
        "pos0": ptn(pos_emb[0]),
        "bqs": ptn(bq * ISQ),
        "bv": np.ascontiguousarray(bv.reshape(CT, 128).T),
        "mask32": mask,
        "ident": np.eye(128, dtype=np.float32).astype(bf),
        "identf": np.eye(128, dtype=np.float32),
    }


def kernel(x, pos_emb, Wq, bq, Wk, bk, Wv, bv, num_heads):
    assert int(num_heads) == NH
    x = np.asarray(x, dtype=np.float32).reshape(B, C, HW)
    if "nc" not in _CACHE:
        _CACHE["nc"] = _build_nc()
    nc = _CACHE["nc"]
    shared = _prep(np.asarray(pos_emb, np.float32), np.asarray(Wq, np.float32),
                   np.asarray(bq, np.float32), np.asarray(Wk, np.float32),
                   np.asarray(bk, np.float32), np.asarray(Wv, np.float32),
                   np.asarray(bv, np.float32))
    in_maps = []
    for i in range(NCORES):
        m = dict(shared)
        m["xs"] = np.ascontiguousarray(x[i * BPC:(i + 1) * BPC])
        in_maps.append(m)
    res = run_bass_kernel_spmd(nc, in_maps, list(range(NCORES)))
    out = np.concatenate([res.results[i]["out"] for i in range(NCORES)], axis=0)
    return out.astype(np.float32)
